# revision 16
# baseline (speedup 1.0000x reference)
"""Trainium2 Bass kernel for attention-weight computation.

Computes attn = softmax(encoder_outputs @ hidden) over seq_len=65536,
returning shape (1, 1, 65536) float32.

Distribution: encoder_outputs [65536, 1024] is sharded by rows across 8
NeuronCores (8192 rows each).  The host casts each slice to fp16 (validated
offline: rel L2 err 3.4e-5 on the softmax output) and pre-arranges it
tile-major so every streamed DMA is a fully contiguous 1 MiB block:
dram row (c*128 + p), col (j*512 + n) holds E[c*512 + n, j*128 + p] for
tile c, h-chunk j, partition p.

Each tile's 512 scores accumulate over the 8 h-chunks into a [1, 512]
PSUM row (hidden chunk = [128,1] fp16 stationary, E^T tile = [128,512]
fp16 moving; all stream DMAs ride one HWDGE ring so tiles land in order).
Flash-softmax stats (m_c, m_c + ln S_c) are computed per tile while later
tiles stream.  One tiny AllGather of the 8x[16,2] stat rows (preceded by
a dummy warm-up collective that keeps the ncfw poll loop hot) lets every
core normalize globally: attn = exps * exp(m_c - g - ln S_tot).
"""

import numpy as np

S_TOTAL = 65536
H = 1024
N_CORES = 8
S_PER = S_TOTAL // N_CORES  # 8192 rows per core
P = 128                     # SBUF partitions
HC = H // P                 # 8 h-chunks
SEG = 512                   # seq columns per tile / PSUM row
N_SC = S_PER // SEG         # 16 tiles == 16 score segments

_CACHE: dict = {}


def _build_module():
    import concourse.bacc as bacc
    import concourse.mybir as mybir
    import concourse.tile as tile

    fp32 = mybir.dt.float32
    fp16 = mybir.dt.float16
    AX = mybir.AxisListType.X
    ALL_CORES = [list(range(N_CORES))]
    Act = mybir.ActivationFunctionType
    MUL = mybir.AluOpType.mult

    nc = bacc.Bacc(
        "TRN2",
        target_bir_lowering=False,
        debug=False,
        enable_asserts=False,
        num_devices=N_CORES,
    )

    # et row (c*128+p), col (j*512+n) = E[c*512+n, j*128+p]
    et = nc.dram_tensor("et", [N_SC * P, HC * SEG], fp16, kind="ExternalInput").ap()
    hc = nc.dram_tensor("hc", [P, HC], fp16, kind="ExternalInput").ap()
    out = nc.dram_tensor("out", [S_PER], fp32, kind="ExternalOutput").ap()

    with tile.TileContext(nc) as tc:
        with (
            tc.tile_pool(name="stream", bufs=6) as stream_pool,
            tc.tile_pool(name="persist", bufs=1) as persist_pool,
            tc.tile_pool(name="small", bufs=1) as small_pool,
            tc.tile_pool(name="psum", bufs=1, space="PSUM") as psum_pool,
            tc.tile_pool(name="dram", bufs=1, space="DRAM") as dram_pool,
        ):
            hid = small_pool.tile([P, HC], fp16)
            nc.gpsimd.dma_start(out=hid, in_=hc)
            ones = small_pool.tile([1, N_SC], fp32)
            nc.vector.memset(ones, 1.0)

            # dummy warm-up collective: keeps the ncfw poll loop busy so the
            # real AllGather's trigger is picked up faster; runs during the
            # stream, result unused
            warm_in = dram_pool.tile([8], fp32)
            warm_out = dram_pool.tile([N_CORES, 8], fp32)
            warm_src = small_pool.tile([1, 8], fp32)
            nc.vector.memset(warm_src, 0.0)
            nc.scalar.dma_start(out=warm_in, in_=warm_src)
            nc.gpsimd.collective_compute(
                "AllGather",
                mybir.AluOpType.bypass,
                replica_groups=ALL_CORES,
                ins=[warm_in.opt()],
                outs=[warm_out.opt()],
            )

            exps_row = persist_pool.tile([1, S_PER], fp32)
            pair_row = small_pool.tile([1, N_SC * 2], fp32)  # m, m+lnS pairs
            negm_row = small_pool.tile([1, N_SC], fp32)
            lns_row = small_pool.tile([1, N_SC], fp32)

            # ---- stream 16 x 1 MiB tiles on one HWDGE ring (in-order);
            # tiles 0 and 15 split in j-halves to shorten pipeline head/tail
            def do_tile(c, halves):
                parts = []
                if halves:
                    for h in range(2):
                        etile = stream_pool.tile(
                            [P, HC * SEG // 2], fp16, tag="eth", bufs=4,
                            name=f"et{c}h{h}",
                        )
                        cols = slice(h * HC * SEG // 2, (h + 1) * HC * SEG // 2)
                        nc.sync.dma_start(
                            out=etile, in_=et[c * P : (c + 1) * P, cols]
                        )
                        parts.append(etile)
                else:
                    etile = stream_pool.tile(
                        [P, HC * SEG], fp16, tag="et", name=f"et{c}"
                    )
                    nc.sync.dma_start(out=etile, in_=et[c * P : (c + 1) * P, :])
                    parts.append(etile)
                ps = psum_pool.tile([1, SEG], fp32, tag="ps", bufs=4, name=f"ps{c}")
                for j in range(HC):
                    src = parts[0] if len(parts) == 1 else parts[j // (HC // 2)]
                    jj = j if len(parts) == 1 else j % (HC // 2)
                    nc.tensor.matmul(
                        ps,
                        hid[:, j : j + 1],
                        src[:, jj * SEG : (jj + 1) * SEG],
                        start=(j == 0),
                        stop=(j == HC - 1),
                    )
                # flash stats for this tile (overlap later tiles' DMA):
                # pair = (m_c, m_c + ln S_c)
                nc.vector.reduce_max(pair_row[:, 2 * c : 2 * c + 1], ps, axis=AX)
                nc.vector.tensor_scalar_mul(
                    negm_row[:, c : c + 1], pair_row[:, 2 * c : 2 * c + 1], -1.0
                )
                nc.scalar.activation(
                    out=exps_row[:, c * SEG : (c + 1) * SEG],
                    in_=ps,
                    func=Act.Exp,
                    bias=negm_row[:, c : c + 1],
                    scale=1.0,
                    accum_out=lns_row[:, c : c + 1],
                )
                nc.scalar.activation(
                    out=lns_row[:, c : c + 1],
                    in_=lns_row[:, c : c + 1],
                    func=Act.Ln,
                    scale=1.0,
                )
                nc.vector.tensor_add(
                    pair_row[:, 2 * c + 1 : 2 * c + 2],
                    pair_row[:, 2 * c : 2 * c + 1],
                    lns_row[:, c : c + 1],
                )

            for c in range(N_SC):
                do_tile(c, halves=(c in (0, N_SC - 1)))

            # ---- one AllGather of the (m, m+lnS) pairs ----
            cc_in = dram_pool.tile([N_SC * 2], fp32)
            cc_out = dram_pool.tile([N_CORES, N_SC * 2], fp32)
            nc.scalar.dma_start(out=cc_in, in_=pair_row)
            nc.gpsimd.collective_compute(
                "AllGather",
                mybir.AluOpType.bypass,
                replica_groups=ALL_CORES,
                ins=[cc_in.opt()],
                outs=[cc_out.opt()],
            )
            pairT = small_pool.tile([N_SC, 2], fp32)
            nc.sync.dma_start(
                out=pairT, in_=cc_in.rearrange("(t two) -> t two", t=N_SC)
            )

            # ---- reshape exps [1,8192]->[16,512] via a DRAM bounce;
            # overlaps the AllGather ----
            sc_dram = dram_pool.tile([S_PER], fp32)
            nc.sync.dma_start(out=sc_dram, in_=exps_row)
            exps16 = persist_pool.tile([N_SC, SEG], fp32)
            nc.sync.dma_start(
                out=exps16, in_=sc_dram.rearrange("(t n) -> t n", t=N_SC)
            )

            # ---- combine: g = max m; S = sum exp((m+lnS) - g);
            # c1 = -g - ln S_tot; attn = exps * exp(m_c + c1) ----
            row = small_pool.tile([1, N_CORES * N_SC * 2], fp32)
            nc.scalar.dma_start(out=row, in_=cc_out.rearrange("a b -> (a b)"))
            rowv = row.rearrange("o (k two) -> o two k", two=2)
            g1 = small_pool.tile([1, 1], fp32)
            nc.vector.reduce_max(g1, rowv[:, 0, :], axis=AX)
            negg1 = small_pool.tile([1, 1], fp32)
            nc.vector.tensor_scalar_mul(negg1, g1, -1.0)
            em = small_pool.tile([1, N_CORES * N_SC], fp32)
            s1 = small_pool.tile([1, 1], fp32)
            nc.scalar.activation(
                out=em,
                in_=rowv[:, 1, :],
                func=Act.Exp,
                bias=negg1,
                scale=1.0,
                accum_out=s1,
            )
            lnS = small_pool.tile([1, 1], fp32)
            nc.scalar.activation(out=lnS, in_=s1, func=Act.Ln, scale=1.0)
            c1 = small_pool.tile([1, 1], fp32)
            nc.vector.tensor_add(c1, g1, lnS)
            nc.vector.tensor_scalar_mul(c1, c1, -1.0)

            # broadcast c1 to the 16 partitions via ones.T @ c1
            bpsum = psum_pool.tile([N_SC, 1], fp32)
            nc.tensor.matmul(bpsum, ones, c1, start=True, stop=True)
            bsc = small_pool.tile([N_SC, 1], fp32)
            nc.scalar.copy(bsc, bpsum)

            # f = exp(m - g - lnS); attn = exps * f
            f16 = small_pool.tile([N_SC, 1], fp32)
            nc.scalar.activation(
                out=f16, in_=pairT[:, 0:1], func=Act.Exp, bias=bsc, scale=1.0
            )
            attn16 = persist_pool.tile([N_SC, SEG], fp32)
            nc.vector.tensor_scalar(
                out=attn16,
                in0=exps16,
                scalar1=f16,
                scalar2=None,
                op0=MUL,
            )
            nc.sync.dma_start(
                out=out.rearrange("(t n) -> t n", t=N_SC), in_=attn16
            )

    nc.compile()
    return nc


def _get_module():
    if "nc" not in _CACHE:
        _CACHE["nc"] = _build_module()
    return _CACHE["nc"]


def _prep_inputs(hidden: np.ndarray, encoder_outputs: np.ndarray):
    hidden = np.asarray(hidden, dtype=np.float32)
    eo = np.asarray(encoder_outputs, dtype=np.float32)
    hcm = np.ascontiguousarray(hidden.reshape(HC, P).T).astype(np.float16)  # [P, HC]
    in_maps = []
    for c in range(N_CORES):
        es = eo[c * S_PER : (c + 1) * S_PER]  # [S_PER, H]
        # [tile, p, j, n] <- [tile*SEG+n, j*P+p]
        ets = (
            es.reshape(N_SC, SEG, HC, P)
            .transpose(0, 3, 2, 1)
            .reshape(N_SC * P, HC * SEG)
            .astype(np.float16)
        )
        in_maps.append({"et": np.ascontiguousarray(ets), "hc": hcm})
    return in_maps


def _run(hidden: np.ndarray, encoder_outputs: np.ndarray, trace: bool = False):
    from concourse.bass_utils import run_bass_kernel_spmd

    nc = _get_module()
    in_maps = _prep_inputs(hidden, encoder_outputs)
    res = run_bass_kernel_spmd(
        nc, in_maps, core_ids=list(range(N_CORES)), trace=trace
    )
    parts = [np.asarray(res.results[c]["out"]).reshape(-1) for c in range(N_CORES)]
    attn = np.concatenate(parts)
    return attn.reshape(1, 1, S_TOTAL).astype(np.float32), res


def kernel(hidden: np.ndarray, encoder_outputs: np.ndarray) -> np.ndarray:
    try:
        out, _ = _run(hidden, encoder_outputs, trace=False)
    except Exception:
        # one retry for transient device/runtime hiccups
        _CACHE.clear()
        out, _ = _run(hidden, encoder_outputs, trace=False)
    return out
